# revision 12
# baseline (speedup 1.0000x reference)
"""Trainium2 Bass kernel for nn_Aggregator (GNN message passing).

h = leaky_relu((ego + segment_sum(ego[src] * w, dst)) @ W.T + b)

Strategy (8 NeuronCores, SPMD single program):
- dst nodes sharded over cores by n % 8; within a core, nodes are bin-packed
  by degree (snake deal) into 98 blocks of <=128 so block loads are equal.
- Self-edges (src=dst, w=1) fold the "+ego" term into the segment sum.
- ego replicated to every core as fp16 [100000, 128]; per-edge rows fetched
  with bulk dma_gather (int16 indices -> 4 banks of 25000 rows, one SWDGE
  queue per bank so descriptor drains overlap).
- Per-(block, bank) slot capacity = max over cores, rounded to 128, so every
  128-edge tile belongs to exactly one block; pad slots gather row 0 of the
  bank with weight 0.
- Per 128-edge tile: S[e, j] = w[e] * (dstl[e] == j) built in ONE DVE
  tensor_scalar op (iota is_equal dstl) mult w; scalars are per-partition APs.
- side.T accumulated in PSUM via matmul(lhsT=G_tile, rhs=S_tile); 4 blocks
  share one PSUM bank tile [128, 512]; single start/stop per bank tile
  (start pending-zeroes the whole 2KB bank region).
- Epilogue per block: sideT -> fp16, matmul with W.T, rank-1 bias matmul,
  leaky_relu = max(x, 0.01x) via ACT scale-copy + DVE max, DMA out fp32.
- Output rows are in (block, slot) order; host unpermutes.

The edge structure (capacities) is computed from the actual inputs at call
time and MAXED over cores so all 8 cores share one static program.
"""

import numpy as np

N_NODES = 100000
D = 128
P = 128
NC = 8
NPC = N_NODES // NC            # 12500 nodes per core
NBLK = -(-NPC // P)            # 98 blocks per core
NBANK = 4
BROWS = N_NODES // NBANK       # 25000 rows per gather bank
CHUNK_BLOCKS = 8
BT_BLOCKS = 4                  # blocks per PSUM bank tile
LEAK = 0.01

TRACE = False                  # set True (e.g. from test.py) to capture HW profile
LAST = {}                      # exec_time_ns etc. after a traced run


# ----------------------------------------------------------------------------
# static structure (shared by all cores), derived from tile counts
# ----------------------------------------------------------------------------

def _build_static(tiles_bq):
    """tiles_bq: int array [NBLK, NBANK] tiles per (block, bank)."""
    chunks = []
    tot_tiles = 0
    b0 = 0
    while b0 < NBLK:
        blocks = list(range(b0, min(b0 + CHUNK_BLOCKS, NBLK)))
        b0 += len(blocks)
        calls = []           # per bank: dict(t_off(chunk-local tiles), n_tiles)
        tile_block = []      # block id per chunk-local tile
        off = 0
        for q in range(NBANK):
            nt = int(sum(int(tiles_bq[b, q]) for b in blocks))
            if nt == 0:
                calls.append(None)
                continue
            calls.append({"q": q, "t_off": off, "n_tiles": nt})
            for b in blocks:
                tile_block.extend([b] * int(tiles_bq[b, q]))
            off += nt
        n_tiles = off
        assert len(tile_block) == n_tiles
        # bank tiles: groups of BT_BLOCKS consecutive blocks
        n_bt = -(-len(blocks) // BT_BLOCKS)
        bt_first = [None] * n_bt
        bt_last = [None] * n_bt
        for t, b in enumerate(tile_block):
            bt = (b - blocks[0]) // BT_BLOCKS
            if bt_first[bt] is None:
                bt_first[bt] = t
            bt_last[bt] = t
        chunks.append({
            "blocks": blocks, "calls": calls, "tiles": n_tiles,
            "tile_block": np.asarray(tile_block, np.int64),
            "n_bt": n_bt, "bt_first": bt_first, "bt_last": bt_last,
            "tile_base": tot_tiles,
        })
        tot_tiles += n_tiles
    return chunks, tot_tiles


def _static_slot_starts(tiles_bq, chunks):
    """global slot start position for each (block, bank)."""
    start = np.zeros((NBLK, NBANK), np.int64)
    for ch in chunks:
        for q in range(NBANK):
            c = ch["calls"][q]
            if c is None:
                continue
            pos = (ch["tile_base"] + c["t_off"]) * P
            for b in ch["blocks"]:
                start[b, q] = pos
                pos += int(tiles_bq[b, q]) * P
    return start


# ----------------------------------------------------------------------------
# host-side data prep
# ----------------------------------------------------------------------------

def _prep(ego, edge_index, edge_weight):
    dst = np.asarray(edge_index[0], np.int64)
    src = np.asarray(edge_index[1], np.int64)
    w = np.asarray(edge_weight, np.float32)
    # append self edges
    selfn = np.arange(N_NODES, dtype=np.int64)
    alldst = np.concatenate([dst, selfn])
    allsrc = np.concatenate([src, selfn])
    allw = np.concatenate([w, np.ones(N_NODES, np.float32)])

    core = alldst % NC          # interleaved so self-edge banks spread evenly
    dloc = alldst // NC

    # Balance node->bin assignment per core (snake deal by descending degree)
    deg = np.bincount(alldst, minlength=N_NODES).reshape(NPC, NC).T  # [NC, NPC]
    bin_of = np.empty((NC, NPC), np.int64)
    idx_in_bin = np.empty((NC, NPC), np.int64)
    ranks = np.arange(NPC)
    rounds = ranks // NBLK
    pos = ranks % NBLK
    bins = np.where(rounds % 2 == 0, pos, NBLK - 1 - pos)
    for c in range(NC):
        order_d = np.argsort(-deg[c], kind="stable")
        bin_of[c, order_d] = bins
        idx_in_bin[c, order_d] = rounds
    assert idx_in_bin.max() < P

    blk = bin_of[core, dloc]
    dsti = idx_in_bin[core, dloc]
    bank = allsrc // BROWS
    key = (core * NBLK + blk) * NBANK + bank   # global group key

    cnt = np.bincount(key, minlength=NC * NBLK * NBANK).reshape(NC, NBLK, NBANK)
    cap = cnt.max(axis=0)                      # [NBLK, NBANK]
    tiles_bq = -(-cap // P)                    # tiles per (block, bank)

    chunks, N_TILES = _build_static(tiles_bq)
    TOT = N_TILES * P
    sstart = _static_slot_starts(tiles_bq, chunks)

    # per-edge target position within its core's slot stream
    order = np.argsort(key, kind="stable")
    key_s = key[order]
    group_sizes = np.bincount(key_s, minlength=NC * NBLK * NBANK)
    group_starts_sorted = np.zeros_like(group_sizes)
    np.cumsum(group_sizes[:-1], out=group_starts_sorted[1:])
    rank = np.arange(len(key_s)) - group_starts_sorted[key_s]
    pos_local = sstart.reshape(-1)[key_s % (NBLK * NBANK)] + rank
    core_s = core[order]

    # per-core slot arrays (pads: idx 0, dsti sentinel, w 0)
    slot_srcloc = np.zeros((NC, TOT), np.int16)
    slot_dstidx = np.full((NC, TOT), -100000, np.int64)
    slot_w = np.zeros((NC, TOT), np.float32)
    srcloc_s = (allsrc[order] - bank[order] * BROWS).astype(np.int16)
    slot_srcloc[core_s, pos_local] = srcloc_s
    slot_dstidx[core_s, pos_local] = dsti[order]
    slot_w[core_s, pos_local] = allw[order]

    # idx wrapped layout [NC, 128, TOT//16]
    arr = slot_srcloc.reshape(NC, TOT // 16, 16)
    idx_wrapped = np.ascontiguousarray(
        np.tile(np.transpose(arr, (0, 2, 1)), (1, 8, 1)))

    # per-tile dstl / w arrays [NC, 128, N_TILES]
    dstl_arr = np.ascontiguousarray(
        slot_dstidx.reshape(NC, N_TILES, P).transpose(0, 2, 1).astype(np.float32))
    w_arr = np.ascontiguousarray(
        slot_w.reshape(NC, N_TILES, P).transpose(0, 2, 1))

    # output unpermute: global node (c, n) -> row bin*128 + idx in core c's out
    row_of_node = (bin_of * P + idx_in_bin)    # [NC, NPC]

    ego_f16 = np.ascontiguousarray(ego.astype(np.float16))
    return chunks, N_TILES, idx_wrapped, dstl_arr, w_arr, ego_f16, row_of_node


# ----------------------------------------------------------------------------
# bass program
# ----------------------------------------------------------------------------

def _build_program(chunks, N_TILES):
    import concourse.mybir as mybir
    from concourse import bacc
    from concourse.tile import TileContext

    dt = mybir.dt
    TOT = N_TILES * P
    nc = bacc.Bacc(None, target_bir_lowering=False, debug=False,
                   num_swdge_queues=4)

    ego_d = nc.dram_tensor("ego", [N_NODES, D], dt.float16, kind="ExternalInput")
    idx_d = nc.dram_tensor("idx", [P, TOT // 16], dt.int16, kind="ExternalInput")
    dstl_d = nc.dram_tensor("dstl", [P, N_TILES], dt.float32, kind="ExternalInput")
    wgt_d = nc.dram_tensor("wgt", [P, N_TILES], dt.float32, kind="ExternalInput")
    wt_d = nc.dram_tensor("wt", [D, D], dt.float16, kind="ExternalInput")
    bias_d = nc.dram_tensor("bias", [1, D], dt.float16, kind="ExternalInput")
    iota_d = nc.dram_tensor("iota", [P, P], dt.float16, kind="ExternalInput")
    out_d = nc.dram_tensor("out", [NBLK * P, D], dt.float32, kind="ExternalOutput")

    with TileContext(nc) as tc:
        with (
            tc.tile_pool(name="const", bufs=1) as cpool,
            tc.tile_pool(name="g", bufs=3) as gpool,
            tc.tile_pool(name="ix", bufs=2) as ixpool,
            tc.tile_pool(name="dw", bufs=2) as dwpool,
            tc.tile_pool(name="s", bufs=64) as spool,
            tc.tile_pool(name="ps", bufs=6, space="PSUM") as pspool,
            tc.tile_pool(name="ps2", bufs=2, space="PSUM") as ps2pool,
            tc.tile_pool(name="eo", bufs=3) as epool,
        ):
            wt_sb = cpool.tile([D, D], dt.float16)
            nc.sync.dma_start(wt_sb[:, :], wt_d[:, :])
            bias_sb = cpool.tile([1, D], dt.float16)
            nc.sync.dma_start(bias_sb[:, :], bias_d[:, :])
            iota_sb = cpool.tile([P, P], dt.float16)
            nc.sync.dma_start(iota_sb[:, :], iota_d[:, :])
            ones_sb = cpool.tile([1, P], dt.float16)
            nc.vector.memset(ones_sb[:, :], 1.0)

            pending = None
            for ci, ch in enumerate(chunks):
                n_tiles = ch["tiles"]
                tb = ch["tile_base"]

                idx_sb = ixpool.tile([P, n_tiles * P // 16], dt.int16, tag="idx")
                nc.sync.dma_start(
                    idx_sb[:, :], idx_d[:, tb * P // 16:(tb + n_tiles) * P // 16])
                dstl_sb = dwpool.tile([P, n_tiles], dt.float32, tag="dstl")
                nc.sync.dma_start(dstl_sb[:, :], dstl_d[:, tb:tb + n_tiles])
                w_sb = dwpool.tile([P, n_tiles], dt.float32, tag="wgt")
                nc.sync.dma_start(w_sb[:, :], wgt_d[:, tb:tb + n_tiles])

                g_slab = gpool.tile([P, n_tiles * D], dt.float16, tag="g")
                subcalls = []
                for q in range(NBANK):
                    c = ch["calls"][q]
                    if c is None:
                        continue
                    t0, nt = c["t_off"], c["n_tiles"]
                    n1 = nt // 2
                    if n1 > 0:
                        subcalls.append((0, q, t0, n1))
                    subcalls.append((1, q, t0 + n1, nt - n1))
                subcalls.sort()
                for _, q, t0, nt in subcalls:
                    s = nt * P
                    out_ap = g_slab[:, t0 * D:(t0 + nt) * D].rearrange(
                        "p (t e) -> p t e", e=D)
                    nc.gpsimd.dma_gather(
                        out_ap,
                        ego_d[q * BROWS:(q + 1) * BROWS, :],
                        idx_sb[:, t0 * P // 16:(t0 * P + s) // 16],
                        s, s, D, elem_step=D, single_packet=False,
                        queue_num=(q + ci) % NBANK,
                    )

                psums = [pspool.tile([P, BT_BLOCKS * P], dt.float32, tag="ps",
                                     name=f"ps_{tb}_{i}")
                         for i in range(ch["n_bt"])]
                blk0 = ch["blocks"][0]
                for t in range(n_tiles):
                    b = int(ch["tile_block"][t])
                    bt = (b - blk0) // BT_BLOCKS
                    col = ((b - blk0) % BT_BLOCKS) * P
                    s_t = spool.tile([P, P], dt.float16, tag="s")
                    nc.vector.tensor_scalar(
                        s_t[:, :], iota_sb[:, :],
                        dstl_sb[:, t:t + 1], w_sb[:, t:t + 1],
                        mybir.AluOpType.is_equal, mybir.AluOpType.mult,
                    )
                    nc.tensor.matmul(
                        out=psums[bt][:, col:col + P],
                        lhsT=g_slab[:, t * D:(t + 1) * D],
                        rhs=s_t[:, :],
                        start=(t == ch["bt_first"][bt]),
                        stop=(t == ch["bt_last"][bt]),
                        skip_group_check=True,
                    )

                def emit_epilogue(ch_e, psums_e):
                    for bt in range(ch_e["n_bt"]):
                        bt_blocks = ch_e["blocks"][bt * BT_BLOCKS:(bt + 1) * BT_BLOCKS]
                        ncols = len(bt_blocks) * P
                        sideT_sb = epool.tile([P, BT_BLOCKS * P], dt.float16,
                                              tag="sideT", name="sideT")
                        nc.scalar.copy(sideT_sb[:, :ncols], psums_e[bt][:, :ncols])
                        for j, b in enumerate(bt_blocks):
                            psum2 = ps2pool.tile([P, D], dt.float32, tag="ps2",
                                                 name="ps2")
                            nc.tensor.matmul(
                                out=psum2[:, :],
                                lhsT=sideT_sb[:, j * P:(j + 1) * P],
                                rhs=wt_sb[:, :],
                                start=True, stop=False, skip_group_check=True,
                            )
                            nc.tensor.matmul(
                                out=psum2[:, :], lhsT=ones_sb[:, :],
                                rhs=bias_sb[:, :],
                                start=False, stop=True, skip_group_check=True,
                            )
                            tmp = epool.tile([P, D], dt.float32, tag="tmp",
                                             name="tmp")
                            nc.scalar.activation(
                                tmp[:, :], psum2[:, :],
                                mybir.ActivationFunctionType.Copy, scale=LEAK)
                            o_sb = epool.tile([P, D], dt.float32, tag="osb",
                                              name="osb")
                            nc.vector.tensor_tensor(
                                o_sb[:, :], psum2[:, :], tmp[:, :],
                                op=mybir.AluOpType.max)
                            nc.scalar.dma_start(
                                out_d[b * P:(b + 1) * P, :], o_sb[:, :])

                if pending is not None:
                    emit_epilogue(*pending)
                pending = (ch, psums)
            emit_epilogue(*pending)

    nc.finalize()
    return nc


# ----------------------------------------------------------------------------
# entry point
# ----------------------------------------------------------------------------

def kernel(ego_embeddings, edge_index, edge_weight, W, b):
    from concourse import bass_utils

    ego = np.asarray(ego_embeddings, np.float32)
    W_np = np.asarray(W, np.float32)
    b_np = np.asarray(b, np.float32)

    (chunks, N_TILES, idx_wrapped, dstl_arr, w_arr, ego_f16,
     row_of_node) = _prep(ego, edge_index, edge_weight)

    nc = _build_program(chunks, N_TILES)

    wt_f16 = np.ascontiguousarray(W_np.T.astype(np.float16))
    bias_f16 = b_np.astype(np.float16)[None, :]
    iota = np.broadcast_to(np.arange(P, dtype=np.float16), (P, P)).copy()

    in_maps = []
    for c in range(NC):
        in_maps.append({
            "ego": ego_f16,
            "idx": idx_wrapped[c],
            "dstl": dstl_arr[c],
            "wgt": w_arr[c],
            "wt": wt_f16,
            "bias": bias_f16,
            "iota": iota,
        })

    res = bass_utils.run_bass_kernel_spmd(
        nc, in_maps, core_ids=list(range(NC)), trace=TRACE)
    LAST["exec_time_ns"] = res.exec_time_ns
    LAST["mean_exec_time_ns"] = res.mean_exec_time_ns
    LAST["slots"] = N_TILES * P
    LAST["entries"] = N_TILES
    LAST["insts"] = res.instructions_and_trace

    out = np.empty((N_NODES, D), np.float32)
    core_nodes = np.arange(N_NODES).reshape(NPC, NC)   # [local, core]
    for c in range(NC):
        out[core_nodes[:, c]] = res.results[c]["out"][row_of_node[c]]
    return out
